# revision 16
# baseline (speedup 1.0000x reference)
"""Trainium2 Bass kernel for nn_AttnMLP: 4x (LayerNorm -> Linear(2048,2048) -> tanh-GELU).

Sharding: data-parallel, batch dim (8 batch elements) across 8 NeuronCores.
Weights (4 x 2048 x 2048) replicated per core, held resident in SBUF one
layer at a time.

Per-core dataflow (token-major layout [tokens, features]):
  for layer l:                      # W_l resident in SBUF (16 MB)
    for token tile i (16 x 128 tokens):
      DMA x tile [128, 2048]
      LN stats via bn_stats/bn_aggr (DVE), normalize in place (DVE)
      PE-transpose x_norm into 16 [128,128] chunks (via PSUM, DVE evacuate)
      k-outer matmul: PSUM[t=128, e=512] += xT_k.T @ WT_k[:, e] (fp32r)
        bias pre-added via a K=1 matmul (ones x bias row)
      GELU (tanh approx) on ScalarE straight out of PSUM
      DMA y tile to DRAM (input of next layer)

LN affine (ln_w, ln_b) is folded into W and b on the host:
  W' = W * ln_w[None, :],  b' = b + W @ ln_b
"""

import sys

sys.path.insert(0, "/opt/trn_rl_repo")

import numpy as np

N_LAYERS = 4
D = 2048  # embedding dim
B = 8  # batch (one element per core)
S = 2048  # sequence length
T = S  # tokens per core
P = 128  # partitions
KC = D // P  # 16 contraction chunks
EC = 4  # output-feature chunks
EW = D // EC  # 512 output features per chunk
LN_EPS = 1e-5


def build(nc, T_tokens=T, n_layers=N_LAYERS, use_f32r=True):
    """Emit the kernel IR into `nc`. Returns None; tensors are declared here."""
    import concourse.bass as bass
    import concourse.mybir as mybir
    import concourse.tile as tile
    from contextlib import ExitStack
    from concourse.masks import make_identity
    from concourse.tile import add_dep_helper

    f32 = mybir.dt.float32
    f32r = mybir.dt.float32r if use_f32r else mybir.dt.float32
    NT = T_tokens // P  # token tiles

    x_d = nc.dram_tensor("x", [T_tokens, D], f32, kind="ExternalInput")
    wt_d = nc.dram_tensor("wt", [n_layers, D, D], f32r, kind="ExternalInput")
    b_d = nc.dram_tensor("b", [n_layers, D], f32r, kind="ExternalInput")
    y_d = nc.dram_tensor("y", [T_tokens, D], f32, kind="ExternalOutput")
    buf0 = nc.dram_tensor("xbuf0", [T_tokens, D], f32)
    buf1 = nc.dram_tensor("xbuf1", [T_tokens, D], f32)

    srcs = [x_d, buf0, buf1, buf0][:n_layers]
    dsts = ([buf0, buf1, buf0][: n_layers - 1] + [y_d]) if n_layers > 1 else [y_d]

    wt_v = wt_d.rearrange("l (kc p) e -> l kc p e", p=P)  # [L, 16, 128, 2048]

    with tile.TileContext(nc) as tc, ExitStack() as ctx:
        singles = ctx.enter_context(tc.tile_pool(name="singles", bufs=1))
        wt_pool = ctx.enter_context(tc.tile_pool(name="wt", bufs=1))
        bias_pool = ctx.enter_context(tc.tile_pool(name="bias", bufs=2))
        x_pool = ctx.enter_context(tc.tile_pool(name="x", bufs=3))
        xt_pool = ctx.enter_context(tc.tile_pool(name="xt", bufs=2))
        y_pool = ctx.enter_context(tc.tile_pool(name="y", bufs=2))
        st_pool = ctx.enter_context(tc.tile_pool(name="st", bufs=4))
        pt_psum = ctx.enter_context(tc.tile_pool(name="ptp", bufs=2, space="PSUM"))
        acc_psum = ctx.enter_context(tc.tile_pool(name="accp", bufs=5, space="PSUM"))
        probe_psum = ctx.enter_context(
            tc.tile_pool(name="probep", bufs=1, space="PSUM")
        )

        ident = singles.tile([P, P], f32)
        make_identity(nc, ident)
        ones_f = singles.tile([1, P], f32)
        nc.vector.memset(ones_f, 1.0)
        ones = singles.tile([1, P], f32r)
        nc.vector.tensor_copy(ones, ones_f)
        eps_t = singles.tile([P, 1], f32)
        nc.vector.memset(eps_t, LN_EPS)

        last_gelus = []
        for l in range(n_layers):
            src = srcs[l].rearrange("(n p) d -> n p d", p=P)
            dst = dsts[l].rearrange("(n p) d -> n p d", p=P)

            wts = []
            for k in range(KC):
                w = wt_pool.tile([P, D], f32r, tag=f"wt{k}")
                nc.sync.dma_start(out=w, in_=wt_v[l, k])
                wts.append(w)
            bias = bias_pool.tile([1, D], f32r, tag="bias")
            nc.sync.dma_start(out=bias, in_=b_d[l].unsqueeze(0))

            # "Probe" transposes: tiny PE instructions that absorb the DMA /
            # ACT semaphore waits so the self-loading fp32r matmuls below
            # never carry more than one sync wait (walrus LW-struct limit).
            wt_probes = []
            for k in range(KC):
                pp = probe_psum.tile([32, 32], f32, tag="probe", name="probe")
                pr = nc.tensor.matmul(
                    out=pp,
                    lhsT=wts[k].bitcast(f32)[0:32, 0:32],
                    rhs=ident[0:32, 0:32],
                    is_transpose=True,
                )
                wt_probes.append(pr.ins)
            ppb = probe_psum.tile([32, 1], f32, tag="probe", name="probe")
            bias_probe = nc.tensor.matmul(
                out=ppb,
                lhsT=bias.bitcast(f32)[0:1, 0:32],
                rhs=ident[0:1, 0:1],
                is_transpose=True,
            ).ins
            act_probe = None
            if last_gelus:
                ppa = probe_psum.tile([32, 32], f32, tag="probe", name="probe")
                act_probe = nc.tensor.matmul(
                    out=ppa,
                    lhsT=ident[0:32, 0:32],
                    rhs=ident[0:32, 0:32],
                    is_transpose=True,
                ).ins
                for g in last_gelus:
                    add_dep_helper(act_probe, g, True, "probe observes ACT sem")
            last_gelus = []

            for i in range(NT):
                xt = x_pool.tile([P, D], f32, tag="x")
                nc.sync.dma_start(out=xt, in_=src[i])

                # --- LayerNorm (token-major: reduce along free dim) ---
                stats = st_pool.tile([P, 4, 6], f32, tag="bnst")
                for g in range(4):
                    nc.vector.bn_stats(
                        out=stats[:, g, :], in_=xt[:, bass.ts(g, 512)]
                    )
                mv = st_pool.tile([P, 2], f32, tag="mv")
                nc.vector.bn_aggr(out=mv, in_=stats)
                rstd = st_pool.tile([P, 1], f32, tag="rstd")
                nc.scalar.activation(
                    out=rstd,
                    in_=mv[:, 1:2],
                    func=mybir.ActivationFunctionType.Sqrt,
                    bias=eps_t,
                    scale=1.0,
                )
                nc.vector.reciprocal(out=rstd, in_=rstd)
                nc.vector.tensor_scalar(
                    out=xt,
                    in0=xt,
                    scalar1=mv[:, 0:1],
                    scalar2=rstd,
                    op0=mybir.AluOpType.subtract,
                    op1=mybir.AluOpType.mult,
                )

                # --- transpose x_norm: 16 chunks of [128,128] via PE ---
                xT = xt_pool.tile([P, KC, P], f32r, tag="xT")
                for g in range(4):
                    pt = pt_psum.tile([P, 4, P], f32, tag="pt")
                    for j in range(4):
                        c = 4 * g + j
                        nc.tensor.matmul(
                            out=pt[:, j, :],
                            lhsT=xt[:, bass.ts(c, P)],
                            rhs=ident,
                            is_transpose=True,
                            start=(j == 0),
                            stop=(j == 3),
                        )
                    nc.vector.tensor_copy(xT[:, bass.ts(g, 4), :], pt)

                # --- matmul + bias + GELU ---
                yt = y_pool.tile([P, D], f32, tag="y")
                accs = [
                    acc_psum.tile([P, EW], f32, tag="acc", name="acc")
                    for _ in range(EC)
                ]
                for e in range(EC):
                    # bias broadcast via K=1 matmul: out[t, e] = 1 * b[e]
                    bm = nc.tensor.matmul(
                        out=accs[e],
                        lhsT=ones,
                        rhs=bias[:, bass.ts(e, EW)],
                        start=True,
                        stop=False,
                    ).ins
                    if i == 0:
                        add_dep_helper(bm, bias_probe, False, "order after probe")
                        if act_probe is not None:
                            add_dep_helper(bm, act_probe, False, "order after probe")
                for k in range(KC):
                    for e in range(EC):
                        mm = nc.tensor.matmul(
                            out=accs[e],
                            lhsT=xT[:, k, :],
                            rhs=wts[k][:, bass.ts(e, EW)],
                            start=False,
                            stop=(k == KC - 1),
                        ).ins
                        if i == 0 and e == 0:
                            add_dep_helper(
                                mm, wt_probes[k], False, "order after probe"
                            )
                for e in range(EC):
                    g = nc.scalar.activation(
                        out=yt[:, bass.ts(e, EW)],
                        in_=accs[e],
                        func=mybir.ActivationFunctionType.Gelu_apprx_tanh,
                    ).ins
                    if i == NT - 1:
                        last_gelus.append(g)
                nc.sync.dma_start(out=dst[i], in_=yt)

    _split_matmul_waits(nc)


def _split_matmul_waits(nc):
    """Walrus encodes fp32/fp32r/transpose matmuls as self-loading LW-struct
    instructions, which accept at most ONE sync-wait command. Tile's wait
    assignment can attach several. Hoist all but one wait of each matmult onto
    standalone EventSemaphore (sequencer) instructions inserted right before
    it on the same engine — semantically identical, codegen-legal."""
    import concourse.mybir as mybir

    skip = ("InstEventSemaphore",)
    n_split = 0
    for fn in nc.m.functions:
        for bb in fn.blocks:
            insts = bb.instructions
            i = 0
            while i < len(insts):
                inst = insts[i]
                if type(inst).__name__ not in skip:
                    si = inst.sync_info
                    waits = list(si.on_wait) if (si and si.on_wait) else []
                    if len(waits) > 1:
                        for j, w in enumerate(waits[:-1]):
                            ev = mybir.InstEventSemaphore(
                                name=f"{inst.name}-hw{j}",
                                engine=inst.engine,
                                sync_info=mybir.SyncInfo(
                                    on_wait=[w], on_update=[]
                                ),
                            )
                            nc.register_instruction(ev, overwrite=True)
                            insts.insert(i, ev)
                            i += 1
                        si.on_wait = [waits[-1]]
                        n_split += 1
                i += 1
    return n_split


_CACHE = {}


def _get_nc():
    if "nc" not in _CACHE:
        import concourse.bass as bass

        nc = bass.Bass("TRN2", target_bir_lowering=False)
        build(nc)
        _CACHE["nc"] = nc
    return _CACHE["nc"]


def _prep_host(x, W, b, ln_w, ln_b):
    """Fold LN affine into weights; pre-transpose W to [L, D_in, D_out]."""
    x = np.ascontiguousarray(np.asarray(x, dtype=np.float32))
    W = np.asarray(W, dtype=np.float32)
    b = np.asarray(b, dtype=np.float32)
    ln_w = np.asarray(ln_w, dtype=np.float32)
    ln_b = np.asarray(ln_b, dtype=np.float32)

    Wf = W * ln_w[:, None, :]  # scale columns (input dim)
    bf = b + np.einsum("led,ld->le", W, ln_b)
    WT = np.ascontiguousarray(Wf.transpose(0, 2, 1))  # [L, D(in), E(out)]
    return x, WT, bf


def run(x, W, b, ln_w, ln_b, trace=False):
    from concourse import bass_utils

    x, WT, bf = _prep_host(x, W, b, ln_w, ln_b)
    nc = _get_nc()
    in_maps = [{"x": x[i], "wt": WT, "b": bf} for i in range(B)]
    res = bass_utils.run_bass_kernel_spmd(
        nc, in_maps, core_ids=list(range(B)), trace=trace
    )
    out = np.stack([res.results[i]["y"] for i in range(B)])
    return out.reshape(B, S, D), res


def kernel(x, W, b, ln_w, ln_b):
    out, _ = run(x, W, b, ln_w, ln_b)
    return out
